# revision 22
# baseline (speedup 1.0000x reference)
"""Trainium2 Bass kernel for EnhancedLocalAttentionWithGQA.

Problem (hardcoded): B=2, L=4096, C=2048, H=16 heads, D=128, G=2 kv groups,
window W=256 with stride 128 (50% overlap).

Key observation: the reference computes NW=31 overlapping windows but the
final output slice [:, :L] keeps only windows 0..15 (16 windows x 256 rows
= 4096 rows).  Window n's output rows [n*256,(n+1)*256) come from queries /
keys / values at input positions [n*128, n*128+256).  So only x positions
0..2175 feed QKV, and each window is an independent 256x256 attention.

Sharding (8 cores): core c -> batch b=c//4, quarter p=c%4, i.e. 4 windows
(global windows 4p..4p+3), input positions [512p, 512p+640), output rows
[1024p, 1024p+1024) of batch b.  No collectives; host concatenates rows.

Per-core pipeline (bf16 matmuls, fp32 PSUM):
  - Startup: x rides the sync+scalar DMA queues in 3 grouped transfers
    (single-chunk strided transfers measured only ~30GB/s; grouped 5-12KB
    lines are fast); weights/biases ride gpsimd.  K/V/Q consume x chunks in
    arrival order with chunk-paired PSUM accumulation, so the PE starts
    ~8us in and is DMA-paced only briefly.
  - Attention: 3-stage pipeline over 32 (window, kv-group, head-pair)
    units, window-major.  S^T on PE, one [128,1024] exp on ACT, colsum via
    ones-matmul, broadcast via K=1 matmul, reciprocal+scale on DVE.
  - As soon as a window's 8 pairs retire, the out-projection rows of that
    window (16 matmuls each) are interleaved into the pipeline, so the PE
    has dense filler while ACT/DVE round-trip, and the out-projection
    effectively starts ~40us early.
All weights are host-pretiled so every DMA is a large contiguous transfer.
"""

import numpy as np
import ml_dtypes

import concourse.bacc as bacc
import concourse.tile as tile
from concourse import mybir
from concourse.bass_utils import run_bass_kernel_spmd

F32 = mybir.dt.float32
BF16 = mybir.dt.bfloat16

B = 2
L = 4096
C = 2048          # embed dim
H = 16            # heads
G = 2             # kv groups
D = 128           # head dim
KV = G * D        # 256
NWL = 4           # windows per core
S = NWL * 128 + 128   # 640 input positions per core
OUT_ROWS = NWL * 256  # 1024 output rows per core
KC = C // 128     # 16 contraction chunks
NT = 4            # out-proj 512-col tiles
SC_OUT = OUT_ROWS // 128
SCALE = 1.0 / float(np.sqrt(D))
N_CORES = 8

# Q-projection head order: pair (g, j) covers heads (g+4j, g+4j+2)
HEAD_ORDER = [h for j in range(4) for g in range(G)
              for h in (g + 4 * j, g + 4 * j + 2)]

_CACHE = {}


def _build():
    nc = bacc.Bacc(None, target_bir_lowering=False)

    # host-pretiled layouts (see kernel() for the numpy side)
    xT_d = nc.dram_tensor("xT", [128, KC, S], BF16, kind="ExternalInput")
    wq_d = nc.dram_tensor("Wq", [H, 128, KC, 128], BF16, kind="ExternalInput")
    wk_d = nc.dram_tensor("Wk", [G, 128, KC, 128], BF16, kind="ExternalInput")
    wv_d = nc.dram_tensor("Wv", [128, KC, KV], BF16, kind="ExternalInput")
    wo_d = nc.dram_tensor("Wo", [NT, 128, KC, 512], BF16, kind="ExternalInput")
    bq_d = nc.dram_tensor("bq", [128, H], F32, kind="ExternalInput")
    bk_d = nc.dram_tensor("bk", [128, G], F32, kind="ExternalInput")
    bv_d = nc.dram_tensor("bv", [KV], F32, kind="ExternalInput")
    bo_d = nc.dram_tensor("bo", [C], F32, kind="ExternalInput")
    out_d = nc.dram_tensor("out", [NT, SC_OUT, 128, 512], F32,
                           kind="ExternalOutput")

    NA = 320  # free split of S=640 (psum bank = 512 f32)

    with tile.TileContext(nc) as tc:
        with (
            tc.tile_pool(name="res", bufs=1) as res,
            tc.tile_pool(name="wqs", bufs=3) as wqs,
            tc.tile_pool(name="wos", bufs=2) as wos,
            tc.tile_pool(name="work", bufs=4) as work,
            tc.tile_pool(name="norm", bufs=3) as norm,
        ):
            # ---------- resident tiles + input DMAs ----------
            # x in 3 grouped DMAs: two on sync, one on the scalar queue
            # (parallel rings), consumed in arrival order
            XG = [(0, 4, "sync"), (4, 10, "sync"), (10, 16, "scalar")]
            xg = [res.tile([128, hi - lo, S], BF16, tag=f"xg{i}",
                           name=f"xg{i}")
                  for i, (lo, hi, _) in enumerate(XG)]

            def xts(kc):
                for i, (lo, hi, _) in enumerate(XG):
                    if lo <= kc < hi:
                        return xg[i][:, kc - lo, :]
                raise AssertionError(kc)

            kw = [res.tile([128, KC, 128], BF16, tag=f"kw{g}", name=f"kw{g}")
                  for g in range(G)]
            wv_t = res.tile([128, KC, KV], BF16, tag="wv", name="wv")
            bq_sb = res.tile([128, H], F32, tag="bq", name="bq")
            bk_sb = res.tile([128, G], F32, tag="bk", name="bk")
            bv_bc = res.tile([128, KV], F32, tag="bvbc", name="bvbc")
            bo_bc = res.tile([128, C], F32, tag="bobc", name="bobc")

            for i, (lo, hi, q) in enumerate(XG):
                eng = nc.sync if q == "sync" else nc.scalar
                eng.dma_start(out=xg[i], in_=xT_d[:, lo:hi, :])
            for g in range(G):
                nc.gpsimd.dma_start(out=kw[g], in_=wk_d[g])
            nc.gpsimd.dma_start(out=wv_t, in_=wv_d[:, :, :])
            nc.gpsimd.dma_start(out=bq_sb, in_=bq_d[:, :])
            nc.gpsimd.dma_start(out=bk_sb, in_=bk_d[:, :])
            nc.gpsimd.dma_start(out=bv_bc,
                                in_=bv_d[:].unsqueeze(0).to_broadcast((128, KV)))
            # bo broadcast (1MB SBUF write) deferred off the startup path

            qp = [res.tile([128, 2, S], BF16, tag=f"qp{i}", name=f"qp{i}")
                  for i in range(8)]

            def q_slot(h):
                g, k = h % G, h // G
                return qp[g * 4 + k // 2][:, k % 2, :]

            kt = [res.tile([128, S], BF16, tag=f"kt{g}", name=f"kt{g}")
                  for g in range(G)]
            vt = [res.tile([128, KV], BF16, tag=f"vt{sc}", name=f"vt{sc}")
                  for sc in range(S // 128)]
            ot = [res.tile([128, OUT_ROWS], BF16, tag=f"ot{h}", name=f"ot{h}")
                  for h in range(H)]

            ones = res.tile([128, 1], BF16, tag="ones", name="ones")
            nc.vector.memset(ones, 1.0)
            ones_r = res.tile([1, 128], BF16, tag="ones_r", name="ones_r")
            nc.vector.memset(ones_r, 1.0)

            # wq prefetch, depth 3 (first two heads ride gpsimd so they do
            # not serialize behind the x stream on sync)
            wq_pending = {}

            def prefetch_wq(h, queue=None):
                t = wqs.tile([128, KC, 128], BF16, tag="wq", name="wq")
                (queue or nc.sync).dma_start(out=t, in_=wq_d[h])
                wq_pending[h] = t

            prefetch_wq(HEAD_ORDER[0], queue=nc.gpsimd)
            prefetch_wq(HEAD_ORDER[1], queue=nc.gpsimd)

            # ---------- projections (chunk-paired, DMA-paced) ----------
            with tc.tile_pool(name="psQ", bufs=1, space="PSUM") as psQ:
                for g in range(G):
                    pa = psQ.tile([128, NA], F32, tag="qa", name="ka")
                    pb = psQ.tile([128, NA], F32, tag="qb", name="kb")
                    for kc in range(KC):
                        nc.tensor.matmul(pa, lhsT=kw[g][:, kc, :],
                                         rhs=xts(kc)[:, 0:NA],
                                         start=(kc == 0), stop=(kc == KC - 1))
                        nc.tensor.matmul(pb, lhsT=kw[g][:, kc, :],
                                         rhs=xts(kc)[:, NA:S],
                                         start=(kc == 0), stop=(kc == KC - 1))
                    nc.scalar.activation(kt[g][:, 0:NA], pa,
                                         mybir.ActivationFunctionType.Identity,
                                         bias=bk_sb[:, g:g + 1])
                    nc.scalar.activation(kt[g][:, NA:S], pb,
                                         mybir.ActivationFunctionType.Identity,
                                         bias=bk_sb[:, g:g + 1])

                for sc in range(S // 128):
                    pv = psQ.tile([128, KV], F32,
                                  tag=("qa" if sc % 2 == 0 else "qb"),
                                  name="pv")
                    for kc in range(KC):
                        nc.tensor.matmul(
                            pv, lhsT=xts(kc)[:, sc * 128:(sc + 1) * 128],
                            rhs=wv_t[:, kc, :],
                            start=(kc == 0), stop=(kc == KC - 1))
                    nc.vector.tensor_add(vt[sc], pv, bv_bc)

                for idx, h in enumerate(HEAD_ORDER):
                    if idx + 2 < H:
                        prefetch_wq(HEAD_ORDER[idx + 2])
                    wq_t = wq_pending.pop(h)
                    pa = psQ.tile([128, NA], F32, tag="qa", name="qa")
                    pb = psQ.tile([128, NA], F32, tag="qb", name="qb")
                    for kc in range(KC):
                        nc.tensor.matmul(pa, lhsT=wq_t[:, kc, :],
                                         rhs=xts(kc)[:, 0:NA],
                                         start=(kc == 0), stop=(kc == KC - 1))
                        nc.tensor.matmul(pb, lhsT=wq_t[:, kc, :],
                                         rhs=xts(kc)[:, NA:S],
                                         start=(kc == 0), stop=(kc == KC - 1))
                    nc.scalar.activation(q_slot(h)[:, 0:NA], pa,
                                         mybir.ActivationFunctionType.Identity,
                                         bias=bq_sb[:, h:h + 1])
                    nc.scalar.activation(q_slot(h)[:, NA:S], pb,
                                         mybir.ActivationFunctionType.Identity,
                                         bias=bq_sb[:, h:h + 1])

            # ---------- attention + early out-projection ----------
            # pairs window-major: window w's ot columns complete after its 8
            # pairs, unlocking out-proj rows 2w, 2w+1 of block nt=0.
            pairs = [(w, g, j) for w in range(NWL)
                     for g in range(G) for j in range(4)]
            NP = len(pairs)
            state = {}
            wo_tiles = {}

            def prefetch_wo(nt):
                if nt == 0:
                    nc.gpsimd.dma_start(
                        out=bo_bc,
                        in_=bo_d[:].unsqueeze(0).to_broadcast((128, C)))
                t = wos.tile([128, KC, 512], BF16, tag="wo", name="wo")
                nc.sync.dma_start(out=t, in_=wo_d[nt])
                wo_tiles[nt] = t

            def po_unit(nt, sc):
                po = psB2.tile([128, 512], F32, tag="ob", name="po")
                for fc in range(KC):
                    nc.tensor.matmul(
                        po, lhsT=ot[fc][:, sc * 128:(sc + 1) * 128],
                        rhs=wo_tiles[nt][:, fc, :],
                        start=(fc == 0), stop=(fc == KC - 1))
                osb = work.tile([128, 512], F32, tag="osb", name="osb")
                nc.vector.tensor_add(osb, po,
                                     bo_bc[:, nt * 512:(nt + 1) * 512])
                nc.sync.dma_start(out=out_d[nt, sc], in_=osb)

            def stage_a(i):
                w, g, j = pairs[i]
                qpt = qp[g * 4 + j]
                stt = psB.tile([128, 1024], F32, tag="st", name="stt")
                for kc in range(2):
                    nc.tensor.matmul(
                        stt[:, kc * 512:(kc + 1) * 512],
                        lhsT=kt[g][:, (w + kc) * 128:(w + kc + 1) * 128],
                        rhs=qpt[:, :, w * 128:w * 128 + 256],
                        start=True, stop=True)
                pt = work.tile([128, 1024], BF16, tag="pt", name="pt")
                nc.scalar.activation(pt, stt,
                                     mybir.ActivationFunctionType.Exp,
                                     scale=SCALE)
                state[i] = [pt]

            def stage_b(i):
                w, g, j = pairs[i]
                (pt,) = state[i]
                obt = psB2.tile([128, 512], F32, tag="ob", name="obt")
                for kc in range(2):
                    nc.tensor.matmul(
                        obt, lhsT=vt[w + kc][:, g * 128:(g + 1) * 128],
                        rhs=pt[:, kc * 512:(kc + 1) * 512],
                        start=(kc == 0), stop=(kc == 1))
                cs = psB3.tile([1, 512], F32, tag="cb", name="cs")
                for kc in range(2):
                    nc.tensor.matmul(cs, lhsT=ones,
                                     rhs=pt[:, kc * 512:(kc + 1) * 512],
                                     start=(kc == 0), stop=(kc == 1))
                csb = norm.tile([1, 512], BF16, tag="csb", name="csb")
                nc.scalar.copy(csb, cs)
                state[i] = [obt, csb]

            def stage_c(i):
                w, g, j = pairs[i]
                h0, h1 = g + 4 * j, g + 4 * j + 2
                obt, csb = state.pop(i)
                bc = psB3.tile([128, 512], F32, tag="cb", name="bc")
                nc.tensor.matmul(bc, lhsT=ones_r, rhs=csb,
                                 start=True, stop=True)
                bcr = norm.tile([128, 512], F32, tag="bcr", name="bcr")
                nc.vector.reciprocal_approx_fast(out=bcr, in_=bc)
                ws = slice(w * 256, (w + 1) * 256)
                nc.vector.tensor_mul(ot[h0][:, ws], obt[:, 0:256],
                                     bcr[:, 0:256])
                nc.vector.tensor_mul(ot[h1][:, ws], obt[:, 256:512],
                                     bcr[:, 256:512])

            with (
                tc.tile_pool(name="psB", bufs=2, space="PSUM") as psB,
                tc.tile_pool(name="psB2", bufs=2, space="PSUM") as psB2,
                tc.tile_pool(name="psB3", bufs=2, space="PSUM") as psB3,
            ):
                prefetch_wo(0)
                for i in range(NP + 2):
                    if 1 <= i < NP + 1:
                        stage_b(i - 1)
                    if i < NP:
                        stage_a(i)
                    if 2 <= i:
                        stage_c(i - 2)
                        if (i - 2) % 8 == 7:       # window w fully retired
                            w = (i - 2) // 8
                            po_unit(0, 2 * w)
                            if w == 0:
                                prefetch_wo(1)
                            po_unit(0, 2 * w + 1)

                # remaining out-proj blocks
                for nt in range(1, NT):
                    if nt + 1 < NT:
                        prefetch_wo(nt + 1)
                    for sc in range(SC_OUT):
                        po_unit(nt, sc)

    nc.compile()
    return nc


def _get_nc():
    if "nc" not in _CACHE:
        _CACHE["nc"] = _build()
    return _CACHE["nc"]


def _prep_weights(Wq, bq, Wk, bk, Wv, bv, Wo, bo):
    bf16 = ml_dtypes.bfloat16
    f32 = lambda a: np.ascontiguousarray(np.asarray(a, dtype=np.float32))
    wq = np.asarray(Wq, np.float32).reshape(KC, 128, H, 128)
    wq = np.ascontiguousarray(wq.transpose(2, 1, 0, 3)).astype(bf16)  # (H,p,kc,f)
    wk = np.asarray(Wk, np.float32).reshape(KC, 128, G, 128)
    wk = np.ascontiguousarray(wk.transpose(2, 1, 0, 3)).astype(bf16)  # (G,p,kc,f)
    wv = np.asarray(Wv, np.float32).reshape(KC, 128, KV)
    wv = np.ascontiguousarray(wv.transpose(1, 0, 2)).astype(bf16)     # (p,kc,f)
    wo = np.asarray(Wo, np.float32).reshape(KC, 128, NT, 512)
    wo = np.ascontiguousarray(wo.transpose(2, 1, 0, 3)).astype(bf16)  # (NT,p,kc,f)
    bq2 = np.ascontiguousarray(f32(bq).reshape(H, 128).T)   # (p, h)
    bk2 = np.ascontiguousarray(f32(bk).reshape(G, 128).T)   # (p, g)
    return {
        "Wq": wq, "Wk": wk, "Wv": wv, "Wo": wo,
        "bq": bq2, "bk": bk2, "bv": f32(bv), "bo": f32(bo),
    }


def _prep_in_maps(x, weights):
    bf16 = ml_dtypes.bfloat16
    in_maps = []
    for c in range(N_CORES):
        b, p = divmod(c, 4)
        xs = np.asarray(x[b, 512 * p:512 * p + S, :], np.float32)
        xT = np.ascontiguousarray(xs.T.reshape(KC, 128, S).transpose(1, 0, 2))
        in_maps.append(dict(weights, xT=xT.astype(bf16)))
    return in_maps


def kernel(x, Wq, bq, Wk, bk, Wv, bv, Wo, bo, **_):
    x = np.asarray(x, dtype=np.float32)
    weights = _prep_weights(Wq, bq, Wk, bk, Wv, bv, Wo, bo)
    in_maps = _prep_in_maps(x, weights)

    nc = _get_nc()
    res = run_bass_kernel_spmd(nc, in_maps, core_ids=list(range(N_CORES)))

    out = np.empty((B, L, C), dtype=np.float32)
    for c in range(N_CORES):
        b, p = divmod(c, 4)
        blk = res.results[c]["out"]  # (NT, SC_OUT, 128, 512)
        rows = blk.transpose(1, 2, 0, 3).reshape(OUT_ROWS, C)
        out[b, 1024 * p:1024 * p + OUT_ROWS, :] = rows
    return out


# revision 23
# speedup vs baseline: 1.0260x; 1.0260x over previous
"""Trainium2 Bass kernel for EnhancedLocalAttentionWithGQA.

Problem (hardcoded): B=2, L=4096, C=2048, H=16 heads, D=128, G=2 kv groups,
window W=256 with stride 128 (50% overlap).

Key observation: the reference computes NW=31 overlapping windows but the
final output slice [:, :L] keeps only windows 0..15 (16 windows x 256 rows
= 4096 rows).  Window n's output rows [n*256,(n+1)*256) come from queries /
keys / values at input positions [n*128, n*128+256).  So only x positions
0..2175 feed QKV, and each window is an independent 256x256 attention.

Sharding (8 cores): core c -> batch b=c//4, quarter p=c%4, i.e. 4 windows
(global windows 4p..4p+3), input positions [512p, 512p+640), output rows
[1024p, 1024p+1024) of batch b.  No collectives; host concatenates rows.

Per-core pipeline (bf16 matmuls, fp32 PSUM):
  - Startup: x rides the sync+scalar DMA queues in 3 grouped transfers
    (single-chunk strided transfers measured only ~30GB/s; grouped 5-12KB
    lines are fast); weights/biases ride gpsimd.  K/V/Q consume x chunks in
    arrival order with chunk-paired PSUM accumulation, so the PE starts
    ~8us in and is DMA-paced only briefly.
  - Attention: 3-stage pipeline over 32 (window, kv-group, head-pair)
    units, window-major.  S^T on PE, one [128,1024] exp on ACT, colsum via
    ones-matmul, broadcast via K=1 matmul, reciprocal+scale on DVE.
  - As soon as a window's 8 pairs retire, the out-projection rows of that
    window (16 matmuls each) are interleaved into the pipeline, so the PE
    has dense filler while ACT/DVE round-trip, and the out-projection
    effectively starts ~40us early.
All weights are host-pretiled so every DMA is a large contiguous transfer.
"""

import numpy as np
import ml_dtypes

import concourse.bacc as bacc
import concourse.tile as tile
from concourse import mybir
from concourse.bass_utils import run_bass_kernel_spmd

F32 = mybir.dt.float32
BF16 = mybir.dt.bfloat16

B = 2
L = 4096
C = 2048          # embed dim
H = 16            # heads
G = 2             # kv groups
D = 128           # head dim
KV = G * D        # 256
NWL = 4           # windows per core
S = NWL * 128 + 128   # 640 input positions per core
OUT_ROWS = NWL * 256  # 1024 output rows per core
KC = C // 128     # 16 contraction chunks
NT = 4            # out-proj 512-col tiles
SC_OUT = OUT_ROWS // 128
SCALE = 1.0 / float(np.sqrt(D))
N_CORES = 8

# Q-projection head order: pair (g, j) covers heads (g+4j, g+4j+2)
HEAD_ORDER = [h for j in range(4) for g in range(G)
              for h in (g + 4 * j, g + 4 * j + 2)]

_CACHE = {}


def _build():
    nc = bacc.Bacc(None, target_bir_lowering=False)

    # host-pretiled layouts (see kernel() for the numpy side)
    xT_d = nc.dram_tensor("xT", [128, KC, S], BF16, kind="ExternalInput")
    wq_d = nc.dram_tensor("Wq", [H, 128, KC, 128], BF16, kind="ExternalInput")
    wk_d = nc.dram_tensor("Wk", [G, 128, KC, 128], BF16, kind="ExternalInput")
    wv_d = nc.dram_tensor("Wv", [128, KC, KV], BF16, kind="ExternalInput")
    wo_d = nc.dram_tensor("Wo", [NT, 128, KC, 512], BF16, kind="ExternalInput")
    bq_d = nc.dram_tensor("bq", [128, H], F32, kind="ExternalInput")
    bk_d = nc.dram_tensor("bk", [128, G], F32, kind="ExternalInput")
    bv_d = nc.dram_tensor("bv", [KV], F32, kind="ExternalInput")
    bo_d = nc.dram_tensor("bo", [C], F32, kind="ExternalInput")
    out_d = nc.dram_tensor("out", [NT, SC_OUT, 128, 512], F32,
                           kind="ExternalOutput")

    NA = 320  # free split of S=640 (psum bank = 512 f32)

    with tile.TileContext(nc) as tc:
        with (
            tc.tile_pool(name="res", bufs=1) as res,
            tc.tile_pool(name="wqs", bufs=3) as wqs,
            tc.tile_pool(name="wos", bufs=2) as wos,
            tc.tile_pool(name="work", bufs=4) as work,
            tc.tile_pool(name="norm", bufs=3) as norm,
        ):
            # ---------- resident tiles + input DMAs ----------
            # x in 3 grouped DMAs: two on sync, one on the scalar queue
            # (parallel rings), consumed in arrival order
            XG = [(0, 4, "sync"), (4, 10, "sync"), (10, 16, "scalar")]
            xg = [res.tile([128, hi - lo, S], BF16, tag=f"xg{i}",
                           name=f"xg{i}")
                  for i, (lo, hi, _) in enumerate(XG)]

            def xts(kc):
                for i, (lo, hi, _) in enumerate(XG):
                    if lo <= kc < hi:
                        return xg[i][:, kc - lo, :]
                raise AssertionError(kc)

            kw = [res.tile([128, KC, 128], BF16, tag=f"kw{g}", name=f"kw{g}")
                  for g in range(G)]
            wv_t = res.tile([128, KC, KV], BF16, tag="wv", name="wv")
            bq_sb = res.tile([128, H], F32, tag="bq", name="bq")
            bk_sb = res.tile([128, G], F32, tag="bk", name="bk")
            bv_bc = res.tile([128, KV], F32, tag="bvbc", name="bvbc")
            bo_bc = res.tile([128, C], F32, tag="bobc", name="bobc")

            # tiny bias DMAs FIRST on gpsimd (behind big transfers they
            # arrive 25us late and stall the first ACT drains)
            nc.gpsimd.dma_start(out=bq_sb, in_=bq_d[:, :])
            nc.gpsimd.dma_start(out=bk_sb, in_=bk_d[:, :])
            nc.gpsimd.dma_start(out=bv_bc,
                                in_=bv_d[:].unsqueeze(0).to_broadcast((128, KV)))
            for i, (lo, hi, q) in enumerate(XG):
                eng = nc.sync if q == "sync" else nc.scalar
                eng.dma_start(out=xg[i], in_=xT_d[:, lo:hi, :])
            nc.scalar.dma_start(out=wv_t, in_=wv_d[:, :, :])
            for g in range(G):
                nc.gpsimd.dma_start(out=kw[g], in_=wk_d[g])
            # bo broadcast (1MB SBUF write) deferred off the startup path

            qp = [res.tile([128, 2, S], BF16, tag=f"qp{i}", name=f"qp{i}")
                  for i in range(8)]

            def q_slot(h):
                g, k = h % G, h // G
                return qp[g * 4 + k // 2][:, k % 2, :]

            kt = [res.tile([128, S], BF16, tag=f"kt{g}", name=f"kt{g}")
                  for g in range(G)]
            vt = [res.tile([128, KV], BF16, tag=f"vt{sc}", name=f"vt{sc}")
                  for sc in range(S // 128)]
            ot = [res.tile([128, OUT_ROWS], BF16, tag=f"ot{h}", name=f"ot{h}")
                  for h in range(H)]

            ones = res.tile([128, 1], BF16, tag="ones", name="ones")
            nc.vector.memset(ones, 1.0)
            ones_r = res.tile([1, 128], BF16, tag="ones_r", name="ones_r")
            nc.vector.memset(ones_r, 1.0)

            # wq prefetch, depth 3 (first two heads ride gpsimd so they do
            # not serialize behind the x stream on sync)
            wq_pending = {}

            def prefetch_wq(h, queue=None):
                t = wqs.tile([128, KC, 128], BF16, tag="wq", name="wq")
                (queue or nc.sync).dma_start(out=t, in_=wq_d[h])
                wq_pending[h] = t

            prefetch_wq(HEAD_ORDER[0], queue=nc.gpsimd)
            prefetch_wq(HEAD_ORDER[1], queue=nc.gpsimd)
            WQ_QUEUES = {}
            for _i in range(2, H):
                WQ_QUEUES[HEAD_ORDER[_i]] = nc.sync if _i % 2 == 0 else nc.scalar

            # ---------- projections (chunk-paired, DMA-paced) ----------
            with tc.tile_pool(name="psQ", bufs=2, space="PSUM") as psQ:
                KORD = list(range(0, 4)) + list(range(10, 16)) + list(range(4, 10))
                for g in range(G):
                    pa = psQ.tile([128, NA], F32, tag="qa", name="ka")
                    pb = psQ.tile([128, NA], F32, tag="qb", name="kb")
                    for ki, kc in enumerate(KORD):
                        nc.tensor.matmul(pa, lhsT=kw[g][:, kc, :],
                                         rhs=xts(kc)[:, 0:NA],
                                         start=(ki == 0), stop=(ki == KC - 1))
                        nc.tensor.matmul(pb, lhsT=kw[g][:, kc, :],
                                         rhs=xts(kc)[:, NA:S],
                                         start=(ki == 0), stop=(ki == KC - 1))
                    nc.scalar.activation(kt[g][:, 0:NA], pa,
                                         mybir.ActivationFunctionType.Identity,
                                         bias=bk_sb[:, g:g + 1])
                    nc.scalar.activation(kt[g][:, NA:S], pb,
                                         mybir.ActivationFunctionType.Identity,
                                         bias=bk_sb[:, g:g + 1])

                for sc in range(S // 128):
                    pv = psQ.tile([128, KV], F32,
                                  tag=("qa" if sc % 2 == 0 else "qb"),
                                  name="pv")
                    for kc in range(KC):
                        nc.tensor.matmul(
                            pv, lhsT=xts(kc)[:, sc * 128:(sc + 1) * 128],
                            rhs=wv_t[:, kc, :],
                            start=(kc == 0), stop=(kc == KC - 1))
                    nc.vector.tensor_add(vt[sc], pv, bv_bc)

                for idx, h in enumerate(HEAD_ORDER):
                    if idx + 2 < H:
                        hh = HEAD_ORDER[idx + 2]
                        prefetch_wq(hh, queue=WQ_QUEUES[hh])
                    wq_t = wq_pending.pop(h)
                    pa = psQ.tile([128, NA], F32, tag="qa", name="qa")
                    pb = psQ.tile([128, NA], F32, tag="qb", name="qb")
                    for kc in range(KC):
                        nc.tensor.matmul(pa, lhsT=wq_t[:, kc, :],
                                         rhs=xts(kc)[:, 0:NA],
                                         start=(kc == 0), stop=(kc == KC - 1))
                        nc.tensor.matmul(pb, lhsT=wq_t[:, kc, :],
                                         rhs=xts(kc)[:, NA:S],
                                         start=(kc == 0), stop=(kc == KC - 1))
                    nc.scalar.activation(q_slot(h)[:, 0:NA], pa,
                                         mybir.ActivationFunctionType.Identity,
                                         bias=bq_sb[:, h:h + 1])
                    nc.scalar.activation(q_slot(h)[:, NA:S], pb,
                                         mybir.ActivationFunctionType.Identity,
                                         bias=bq_sb[:, h:h + 1])

            # ---------- attention + early out-projection ----------
            # pairs window-major: window w's ot columns complete after its 8
            # pairs, unlocking out-proj rows 2w, 2w+1 of block nt=0.
            pairs = [(w, g, j) for w in range(NWL)
                     for g in range(G) for j in range(4)]
            NP = len(pairs)
            state = {}
            wo_tiles = {}

            def prefetch_wo(nt):
                if nt == 0:
                    nc.gpsimd.dma_start(
                        out=bo_bc,
                        in_=bo_d[:].unsqueeze(0).to_broadcast((128, C)))
                t = wos.tile([128, KC, 512], BF16, tag="wo", name="wo")
                nc.sync.dma_start(out=t, in_=wo_d[nt])
                wo_tiles[nt] = t

            def po_unit(nt, sc):
                po = psB2.tile([128, 512], F32, tag="ob", name="po")
                for fc in range(KC):
                    nc.tensor.matmul(
                        po, lhsT=ot[fc][:, sc * 128:(sc + 1) * 128],
                        rhs=wo_tiles[nt][:, fc, :],
                        start=(fc == 0), stop=(fc == KC - 1))
                osb = work.tile([128, 512], F32, tag="osb", name="osb")
                nc.vector.tensor_add(osb, po,
                                     bo_bc[:, nt * 512:(nt + 1) * 512])
                nc.sync.dma_start(out=out_d[nt, sc], in_=osb)

            def stage_a(i):
                w, g, j = pairs[i]
                qpt = qp[g * 4 + j]
                stt = psB.tile([128, 1024], F32, tag="st", name="stt")
                for kc in range(2):
                    nc.tensor.matmul(
                        stt[:, kc * 512:(kc + 1) * 512],
                        lhsT=kt[g][:, (w + kc) * 128:(w + kc + 1) * 128],
                        rhs=qpt[:, :, w * 128:w * 128 + 256],
                        start=True, stop=True)
                pt = work.tile([128, 1024], BF16, tag="pt", name="pt")
                nc.scalar.activation(pt, stt,
                                     mybir.ActivationFunctionType.Exp,
                                     scale=SCALE)
                state[i] = [pt]

            def stage_b(i):
                w, g, j = pairs[i]
                (pt,) = state[i]
                obt = psB2.tile([128, 512], F32, tag="ob", name="obt")
                for kc in range(2):
                    nc.tensor.matmul(
                        obt, lhsT=vt[w + kc][:, g * 128:(g + 1) * 128],
                        rhs=pt[:, kc * 512:(kc + 1) * 512],
                        start=(kc == 0), stop=(kc == 1))
                cs = psB3.tile([1, 512], F32, tag="cb", name="cs")
                for kc in range(2):
                    nc.tensor.matmul(cs, lhsT=ones,
                                     rhs=pt[:, kc * 512:(kc + 1) * 512],
                                     start=(kc == 0), stop=(kc == 1))
                csb = norm.tile([1, 512], BF16, tag="csb", name="csb")
                nc.scalar.copy(csb, cs)
                state[i] = [obt, csb]

            def stage_c(i):
                w, g, j = pairs[i]
                h0, h1 = g + 4 * j, g + 4 * j + 2
                obt, csb = state.pop(i)
                bc = psB3.tile([128, 512], F32, tag="cb", name="bc")
                nc.tensor.matmul(bc, lhsT=ones_r, rhs=csb,
                                 start=True, stop=True)
                bcr = norm.tile([128, 512], F32, tag="bcr", name="bcr")
                nc.vector.reciprocal_approx_fast(out=bcr, in_=bc)
                ws = slice(w * 256, (w + 1) * 256)
                nc.vector.tensor_mul(ot[h0][:, ws], obt[:, 0:256],
                                     bcr[:, 0:256])
                nc.vector.tensor_mul(ot[h1][:, ws], obt[:, 256:512],
                                     bcr[:, 256:512])

            with (
                tc.tile_pool(name="psB", bufs=2, space="PSUM") as psB,
                tc.tile_pool(name="psB2", bufs=2, space="PSUM") as psB2,
                tc.tile_pool(name="psB3", bufs=2, space="PSUM") as psB3,
            ):
                prefetch_wo(0)
                for i in range(NP + 4):
                    if 2 <= i < NP + 2:
                        stage_b(i - 2)
                    if i < NP:
                        stage_a(i)
                    if 4 <= i:
                        stage_c(i - 4)
                        if (i - 4) % 8 == 7:       # window w fully retired
                            w = (i - 4) // 8
                            po_unit(0, 2 * w)
                            if w == 0:
                                prefetch_wo(1)
                            po_unit(0, 2 * w + 1)

                # remaining out-proj blocks
                for nt in range(1, NT):
                    if nt + 1 < NT:
                        prefetch_wo(nt + 1)
                    for sc in range(SC_OUT):
                        po_unit(nt, sc)

    nc.compile()
    return nc


def _get_nc():
    if "nc" not in _CACHE:
        _CACHE["nc"] = _build()
    return _CACHE["nc"]


def _prep_weights(Wq, bq, Wk, bk, Wv, bv, Wo, bo):
    bf16 = ml_dtypes.bfloat16
    f32 = lambda a: np.ascontiguousarray(np.asarray(a, dtype=np.float32))
    wq = np.asarray(Wq, np.float32).reshape(KC, 128, H, 128)
    wq = np.ascontiguousarray(wq.transpose(2, 1, 0, 3)).astype(bf16)  # (H,p,kc,f)
    wk = np.asarray(Wk, np.float32).reshape(KC, 128, G, 128)
    wk = np.ascontiguousarray(wk.transpose(2, 1, 0, 3)).astype(bf16)  # (G,p,kc,f)
    wv = np.asarray(Wv, np.float32).reshape(KC, 128, KV)
    wv = np.ascontiguousarray(wv.transpose(1, 0, 2)).astype(bf16)     # (p,kc,f)
    wo = np.asarray(Wo, np.float32).reshape(KC, 128, NT, 512)
    wo = np.ascontiguousarray(wo.transpose(2, 1, 0, 3)).astype(bf16)  # (NT,p,kc,f)
    bq2 = np.ascontiguousarray(f32(bq).reshape(H, 128).T)   # (p, h)
    bk2 = np.ascontiguousarray(f32(bk).reshape(G, 128).T)   # (p, g)
    return {
        "Wq": wq, "Wk": wk, "Wv": wv, "Wo": wo,
        "bq": bq2, "bk": bk2, "bv": f32(bv), "bo": f32(bo),
    }


def _prep_in_maps(x, weights):
    bf16 = ml_dtypes.bfloat16
    in_maps = []
    for c in range(N_CORES):
        b, p = divmod(c, 4)
        xs = np.asarray(x[b, 512 * p:512 * p + S, :], np.float32)
        xT = np.ascontiguousarray(xs.T.reshape(KC, 128, S).transpose(1, 0, 2))
        in_maps.append(dict(weights, xT=xT.astype(bf16)))
    return in_maps


def kernel(x, Wq, bq, Wk, bk, Wv, bv, Wo, bo, **_):
    x = np.asarray(x, dtype=np.float32)
    weights = _prep_weights(Wq, bq, Wk, bk, Wv, bv, Wo, bo)
    in_maps = _prep_in_maps(x, weights)

    nc = _get_nc()
    res = run_bass_kernel_spmd(nc, in_maps, core_ids=list(range(N_CORES)))

    out = np.empty((B, L, C), dtype=np.float32)
    for c in range(N_CORES):
        b, p = divmod(c, 4)
        blk = res.results[c]["out"]  # (NT, SC_OUT, 128, 512)
        rows = blk.transpose(1, 2, 0, 3).reshape(OUT_ROWS, C)
        out[b, 1024 * p:1024 * p + OUT_ROWS, :] = rows
    return out


# revision 24
# speedup vs baseline: 1.0453x; 1.0188x over previous
"""Trainium2 Bass kernel for EnhancedLocalAttentionWithGQA.

Problem (hardcoded): B=2, L=4096, C=2048, H=16 heads, D=128, G=2 kv groups,
window W=256 with stride 128 (50% overlap).

Key observation: the reference computes NW=31 overlapping windows but the
final output slice [:, :L] keeps only windows 0..15 (16 windows x 256 rows
= 4096 rows).  Window n's output rows [n*256,(n+1)*256) come from queries /
keys / values at input positions [n*128, n*128+256).  So only x positions
0..2175 feed QKV, and each window is an independent 256x256 attention.

Sharding (8 cores): core c -> batch b=c//4, quarter p=c%4, i.e. 4 windows
(global windows 4p..4p+3), input positions [512p, 512p+640), output rows
[1024p, 1024p+1024) of batch b.  No collectives; host concatenates rows.

Per-core pipeline (bf16 matmuls, fp32 PSUM):
  - Startup: x rides the sync+scalar DMA queues in 3 grouped transfers
    (single-chunk strided transfers measured only ~30GB/s; grouped 5-12KB
    lines are fast); weights/biases ride gpsimd.  K/V/Q consume x chunks in
    arrival order with chunk-paired PSUM accumulation, so the PE starts
    ~8us in and is DMA-paced only briefly.
  - Attention: 3-stage pipeline over 32 (window, kv-group, head-pair)
    units, window-major.  S^T on PE, one [128,1024] exp on ACT, colsum via
    ones-matmul, broadcast via K=1 matmul, reciprocal+scale on DVE.
  - As soon as a window's 8 pairs retire, the out-projection rows of that
    window (16 matmuls each) are interleaved into the pipeline, so the PE
    has dense filler while ACT/DVE round-trip, and the out-projection
    effectively starts ~40us early.
All weights are host-pretiled so every DMA is a large contiguous transfer.
"""

import numpy as np
import ml_dtypes

import concourse.bacc as bacc
import concourse.tile as tile
from concourse import mybir
from concourse.bass_utils import run_bass_kernel_spmd

F32 = mybir.dt.float32
BF16 = mybir.dt.bfloat16

B = 2
L = 4096
C = 2048          # embed dim
H = 16            # heads
G = 2             # kv groups
D = 128           # head dim
KV = G * D        # 256
NWL = 4           # windows per core
S = NWL * 128 + 128   # 640 input positions per core
OUT_ROWS = NWL * 256  # 1024 output rows per core
KC = C // 128     # 16 contraction chunks
NT = 4            # out-proj 512-col tiles
SC_OUT = OUT_ROWS // 128
SCALE = 1.0 / float(np.sqrt(D))
N_CORES = 8

# Q-projection head order: pair (g, j) covers heads (g+4j, g+4j+2)
HEAD_ORDER = [h for j in range(4) for g in range(G)
              for h in (g + 4 * j, g + 4 * j + 2)]

_CACHE = {}


def _build():
    nc = bacc.Bacc(None, target_bir_lowering=False)

    # host-pretiled layouts (see kernel() for the numpy side)
    xT_d = nc.dram_tensor("xT", [128, KC, S], BF16, kind="ExternalInput")
    wq_d = nc.dram_tensor("Wq", [H, 128, KC, 128], BF16, kind="ExternalInput")
    wk_d = nc.dram_tensor("Wk", [G, 128, KC, 128], BF16, kind="ExternalInput")
    wv_d = nc.dram_tensor("Wv", [128, KC, KV], BF16, kind="ExternalInput")
    wo_d = nc.dram_tensor("Wo", [NT, 128, KC, 512], BF16, kind="ExternalInput")
    bq_d = nc.dram_tensor("bq", [128, H], F32, kind="ExternalInput")
    bk_d = nc.dram_tensor("bk", [128, G], F32, kind="ExternalInput")
    bv_d = nc.dram_tensor("bv", [128, KV], F32, kind="ExternalInput")
    bo_d = nc.dram_tensor("bo", [128, C], F32, kind="ExternalInput")
    out_d = nc.dram_tensor("out", [NT, SC_OUT, 128, 512], F32,
                           kind="ExternalOutput")

    NA = 320  # free split of S=640 (psum bank = 512 f32)

    with tile.TileContext(nc) as tc:
        with (
            tc.tile_pool(name="res", bufs=1) as res,
            tc.tile_pool(name="wqs", bufs=3) as wqs,
            tc.tile_pool(name="wos", bufs=2) as wos,
            tc.tile_pool(name="work", bufs=4) as work,
            tc.tile_pool(name="norm", bufs=3) as norm,
        ):
            # ---------- resident tiles + input DMAs ----------
            # x in 3 grouped DMAs: two on sync, one on the scalar queue
            # (parallel rings), consumed in arrival order
            XG = [(0, 4, "sync"), (4, 10, "sync"), (10, 16, "scalar")]
            xg = [res.tile([128, hi - lo, S], BF16, tag=f"xg{i}",
                           name=f"xg{i}")
                  for i, (lo, hi, _) in enumerate(XG)]

            def xts(kc):
                for i, (lo, hi, _) in enumerate(XG):
                    if lo <= kc < hi:
                        return xg[i][:, kc - lo, :]
                raise AssertionError(kc)

            kw = [res.tile([128, KC, 128], BF16, tag=f"kw{g}", name=f"kw{g}")
                  for g in range(G)]
            wv_t = res.tile([128, KC, KV], BF16, tag="wv", name="wv")
            bq_sb = res.tile([128, H], F32, tag="bq", name="bq")
            bk_sb = res.tile([128, G], F32, tag="bk", name="bk")
            bv_bc = res.tile([128, KV], F32, tag="bvbc", name="bvbc")
            bo_bc = res.tile([128, C], F32, tag="bobc", name="bobc")

            # tiny bias DMAs FIRST on gpsimd (behind big transfers they
            # arrive 25us late and stall the first ACT drains)
            nc.gpsimd.dma_start(out=bq_sb, in_=bq_d[:, :])
            nc.gpsimd.dma_start(out=bk_sb, in_=bk_d[:, :])
            nc.gpsimd.dma_start(out=bv_bc, in_=bv_d[:, :])
            for i, (lo, hi, q) in enumerate(XG):
                eng = nc.sync if q == "sync" else nc.scalar
                eng.dma_start(out=xg[i], in_=xT_d[:, lo:hi, :])
            nc.scalar.dma_start(out=wv_t, in_=wv_d[:, :, :])
            for g in range(G):
                nc.gpsimd.dma_start(out=kw[g], in_=wk_d[g])
            # bo broadcast (1MB SBUF write) deferred off the startup path

            qp = [res.tile([128, 2, S], BF16, tag=f"qp{i}", name=f"qp{i}")
                  for i in range(8)]

            def q_slot(h):
                g, k = h % G, h // G
                return qp[g * 4 + k // 2][:, k % 2, :]

            kt = [res.tile([128, S], BF16, tag=f"kt{g}", name=f"kt{g}")
                  for g in range(G)]
            vt = [res.tile([128, KV], BF16, tag=f"vt{sc}", name=f"vt{sc}")
                  for sc in range(S // 128)]
            ot = [res.tile([128, OUT_ROWS], BF16, tag=f"ot{h}", name=f"ot{h}")
                  for h in range(H)]

            ones = res.tile([128, 1], BF16, tag="ones", name="ones")
            nc.vector.memset(ones, 1.0)
            ones_r = res.tile([1, 128], BF16, tag="ones_r", name="ones_r")
            nc.vector.memset(ones_r, 1.0)

            # wq prefetch, depth 3 (first two heads ride gpsimd so they do
            # not serialize behind the x stream on sync)
            wq_pending = {}

            def prefetch_wq(h, queue=None):
                t = wqs.tile([128, KC, 128], BF16, tag="wq", name="wq")
                (queue or nc.sync).dma_start(out=t, in_=wq_d[h])
                wq_pending[h] = t

            prefetch_wq(HEAD_ORDER[0], queue=nc.gpsimd)
            prefetch_wq(HEAD_ORDER[1], queue=nc.gpsimd)
            WQ_QUEUES = {}
            for _i in range(2, H):
                WQ_QUEUES[HEAD_ORDER[_i]] = nc.sync if _i % 2 == 0 else nc.scalar

            # ---------- projections (chunk-paired, DMA-paced) ----------
            with tc.tile_pool(name="psQ", bufs=2, space="PSUM") as psQ:
                KORD = list(range(0, 4)) + list(range(10, 16)) + list(range(4, 10))
                for g in range(G):
                    pa = psQ.tile([128, NA], F32, tag="qa", name="ka")
                    pb = psQ.tile([128, NA], F32, tag="qb", name="kb")
                    for ki, kc in enumerate(KORD):
                        nc.tensor.matmul(pa, lhsT=kw[g][:, kc, :],
                                         rhs=xts(kc)[:, 0:NA],
                                         start=(ki == 0), stop=(ki == KC - 1))
                        nc.tensor.matmul(pb, lhsT=kw[g][:, kc, :],
                                         rhs=xts(kc)[:, NA:S],
                                         start=(ki == 0), stop=(ki == KC - 1))
                    nc.scalar.activation(kt[g][:, 0:NA], pa,
                                         mybir.ActivationFunctionType.Identity,
                                         bias=bk_sb[:, g:g + 1])
                    nc.scalar.activation(kt[g][:, NA:S], pb,
                                         mybir.ActivationFunctionType.Identity,
                                         bias=bk_sb[:, g:g + 1])

                for sc in range(S // 128):
                    pv = psQ.tile([128, KV], F32,
                                  tag=("qa" if sc % 2 == 0 else "qb"),
                                  name="pv")
                    for kc in range(KC):
                        nc.tensor.matmul(
                            pv, lhsT=xts(kc)[:, sc * 128:(sc + 1) * 128],
                            rhs=wv_t[:, kc, :],
                            start=(kc == 0), stop=(kc == KC - 1))
                    nc.vector.tensor_add(vt[sc], pv, bv_bc)

                for idx, h in enumerate(HEAD_ORDER):
                    if idx + 2 < H:
                        hh = HEAD_ORDER[idx + 2]
                        prefetch_wq(hh, queue=WQ_QUEUES[hh])
                    wq_t = wq_pending.pop(h)
                    pa = psQ.tile([128, NA], F32, tag="qa", name="qa")
                    pb = psQ.tile([128, NA], F32, tag="qb", name="qb")
                    for kc in range(KC):
                        nc.tensor.matmul(pa, lhsT=wq_t[:, kc, :],
                                         rhs=xts(kc)[:, 0:NA],
                                         start=(kc == 0), stop=(kc == KC - 1))
                        nc.tensor.matmul(pb, lhsT=wq_t[:, kc, :],
                                         rhs=xts(kc)[:, NA:S],
                                         start=(kc == 0), stop=(kc == KC - 1))
                    nc.scalar.activation(q_slot(h)[:, 0:NA], pa,
                                         mybir.ActivationFunctionType.Identity,
                                         bias=bq_sb[:, h:h + 1])
                    nc.scalar.activation(q_slot(h)[:, NA:S], pb,
                                         mybir.ActivationFunctionType.Identity,
                                         bias=bq_sb[:, h:h + 1])

            # ---------- attention + early out-projection ----------
            # pairs window-major: window w's ot columns complete after its 8
            # pairs, unlocking out-proj rows 2w, 2w+1 of block nt=0.
            pairs = [(w, g, j) for w in range(NWL)
                     for g in range(G) for j in range(4)]
            NP = len(pairs)
            state = {}
            wo_tiles = {}

            def prefetch_wo(nt):
                if nt == 0:
                    nc.gpsimd.dma_start(out=bo_bc, in_=bo_d[:, :])
                t = wos.tile([128, KC, 512], BF16, tag="wo", name="wo")
                nc.sync.dma_start(out=t, in_=wo_d[nt])
                wo_tiles[nt] = t

            def po_unit(nt, sc):
                po = psB2.tile([128, 512], F32, tag="ob", name="po")
                for fc in range(KC):
                    nc.tensor.matmul(
                        po, lhsT=ot[fc][:, sc * 128:(sc + 1) * 128],
                        rhs=wo_tiles[nt][:, fc, :],
                        start=(fc == 0), stop=(fc == KC - 1))
                osb = work.tile([128, 512], F32, tag="osb", name="osb")
                nc.vector.tensor_add(osb, po,
                                     bo_bc[:, nt * 512:(nt + 1) * 512])
                nc.sync.dma_start(out=out_d[nt, sc], in_=osb)

            def stage_a(i):
                w, g, j = pairs[i]
                qpt = qp[g * 4 + j]
                stt = psB.tile([128, 1024], F32, tag="st", name="stt")
                for kc in range(2):
                    nc.tensor.matmul(
                        stt[:, kc * 512:(kc + 1) * 512],
                        lhsT=kt[g][:, (w + kc) * 128:(w + kc + 1) * 128],
                        rhs=qpt[:, :, w * 128:w * 128 + 256],
                        start=True, stop=True)
                pt = work.tile([128, 1024], BF16, tag="pt", name="pt")
                nc.scalar.activation(pt, stt,
                                     mybir.ActivationFunctionType.Exp,
                                     scale=SCALE)
                state[i] = [pt]

            def stage_b(i):
                w, g, j = pairs[i]
                (pt,) = state[i]
                obt = psB2.tile([128, 512], F32, tag="ob", name="obt")
                for kc in range(2):
                    nc.tensor.matmul(
                        obt, lhsT=vt[w + kc][:, g * 128:(g + 1) * 128],
                        rhs=pt[:, kc * 512:(kc + 1) * 512],
                        start=(kc == 0), stop=(kc == 1))
                cs = psB3.tile([1, 512], F32, tag="cb", name="cs")
                for kc in range(2):
                    nc.tensor.matmul(cs, lhsT=ones,
                                     rhs=pt[:, kc * 512:(kc + 1) * 512],
                                     start=(kc == 0), stop=(kc == 1))
                csb = norm.tile([1, 512], BF16, tag="csb", name="csb")
                nc.scalar.copy(csb, cs)
                state[i] = [obt, csb]

            def stage_c(i):
                w, g, j = pairs[i]
                h0, h1 = g + 4 * j, g + 4 * j + 2
                obt, csb = state.pop(i)
                bc = psB3.tile([128, 512], F32, tag="cb", name="bc")
                nc.tensor.matmul(bc, lhsT=ones_r, rhs=csb,
                                 start=True, stop=True)
                bcr = norm.tile([128, 512], F32, tag="bcr", name="bcr")
                nc.vector.reciprocal_approx_fast(out=bcr, in_=bc)
                ws = slice(w * 256, (w + 1) * 256)
                nc.vector.tensor_mul(ot[h0][:, ws], obt[:, 0:256],
                                     bcr[:, 0:256])
                nc.vector.tensor_mul(ot[h1][:, ws], obt[:, 256:512],
                                     bcr[:, 256:512])

            with (
                tc.tile_pool(name="psB", bufs=2, space="PSUM") as psB,
                tc.tile_pool(name="psB2", bufs=2, space="PSUM") as psB2,
                tc.tile_pool(name="psB3", bufs=2, space="PSUM") as psB3,
            ):
                prefetch_wo(0)
                for i in range(NP + 4):
                    if 2 <= i < NP + 2:
                        stage_b(i - 2)
                    if i < NP:
                        stage_a(i)
                    if 4 <= i:
                        stage_c(i - 4)
                        if (i - 4) % 8 == 7:       # window w fully retired
                            w = (i - 4) // 8
                            po_unit(0, 2 * w)
                            if w == 0:
                                prefetch_wo(1)
                            po_unit(0, 2 * w + 1)

                # remaining out-proj blocks
                for nt in range(1, NT):
                    if nt + 1 < NT:
                        prefetch_wo(nt + 1)
                    for sc in range(SC_OUT):
                        po_unit(nt, sc)

    nc.compile()
    return nc


def _get_nc():
    if "nc" not in _CACHE:
        _CACHE["nc"] = _build()
    return _CACHE["nc"]


def _prep_weights(Wq, bq, Wk, bk, Wv, bv, Wo, bo):
    bf16 = ml_dtypes.bfloat16
    f32 = lambda a: np.ascontiguousarray(np.asarray(a, dtype=np.float32))
    wq = np.asarray(Wq, np.float32).reshape(KC, 128, H, 128)
    wq = np.ascontiguousarray(wq.transpose(2, 1, 0, 3)).astype(bf16)  # (H,p,kc,f)
    wk = np.asarray(Wk, np.float32).reshape(KC, 128, G, 128)
    wk = np.ascontiguousarray(wk.transpose(2, 1, 0, 3)).astype(bf16)  # (G,p,kc,f)
    wv = np.asarray(Wv, np.float32).reshape(KC, 128, KV)
    wv = np.ascontiguousarray(wv.transpose(1, 0, 2)).astype(bf16)     # (p,kc,f)
    wo = np.asarray(Wo, np.float32).reshape(KC, 128, NT, 512)
    wo = np.ascontiguousarray(wo.transpose(2, 1, 0, 3)).astype(bf16)  # (NT,p,kc,f)
    bq2 = np.ascontiguousarray(f32(bq).reshape(H, 128).T)   # (p, h)
    bk2 = np.ascontiguousarray(f32(bk).reshape(G, 128).T)    # (p, g)
    # bv/bo pre-broadcast on host: a [1,N]->[128,N] broadcast DMA lowers to
    # per-element packets (~20us); a contiguous 2D copy is ~2us
    bv2 = np.ascontiguousarray(np.broadcast_to(f32(bv), (128, KV)))
    bo2 = np.ascontiguousarray(np.broadcast_to(f32(bo), (128, C)))
    return {
        "Wq": wq, "Wk": wk, "Wv": wv, "Wo": wo,
        "bq": bq2, "bk": bk2, "bv": bv2, "bo": bo2,
    }


def _prep_in_maps(x, weights):
    bf16 = ml_dtypes.bfloat16
    in_maps = []
    for c in range(N_CORES):
        b, p = divmod(c, 4)
        xs = np.asarray(x[b, 512 * p:512 * p + S, :], np.float32)
        xT = np.ascontiguousarray(xs.T.reshape(KC, 128, S).transpose(1, 0, 2))
        in_maps.append(dict(weights, xT=xT.astype(bf16)))
    return in_maps


def kernel(x, Wq, bq, Wk, bk, Wv, bv, Wo, bo, **_):
    x = np.asarray(x, dtype=np.float32)
    weights = _prep_weights(Wq, bq, Wk, bk, Wv, bv, Wo, bo)
    in_maps = _prep_in_maps(x, weights)

    nc = _get_nc()
    res = run_bass_kernel_spmd(nc, in_maps, core_ids=list(range(N_CORES)))

    out = np.empty((B, L, C), dtype=np.float32)
    for c in range(N_CORES):
        b, p = divmod(c, 4)
        blk = res.results[c]["out"]  # (NT, SC_OUT, 128, 512)
        rows = blk.transpose(1, 2, 0, 3).reshape(OUT_ROWS, C)
        out[b, 1024 * p:1024 * p + OUT_ROWS, :] = rows
    return out


# revision 25
# speedup vs baseline: 1.0568x; 1.0110x over previous
"""Trainium2 Bass kernel for EnhancedLocalAttentionWithGQA.

Problem (hardcoded): B=2, L=4096, C=2048, H=16 heads, D=128, G=2 kv groups,
window W=256 with stride 128 (50% overlap).

Key observation: the reference computes NW=31 overlapping windows but the
final output slice [:, :L] keeps only windows 0..15 (16 windows x 256 rows
= 4096 rows).  Window n's output rows [n*256,(n+1)*256) come from queries /
keys / values at input positions [n*128, n*128+256).  So only x positions
0..2175 feed QKV, and each window is an independent 256x256 attention.

Sharding (8 cores): core c -> batch b=c//4, quarter p=c%4, i.e. 4 windows
(global windows 4p..4p+3), input positions [512p, 512p+640), output rows
[1024p, 1024p+1024) of batch b.  No collectives; host concatenates rows.

Per-core pipeline (bf16 matmuls, fp32 PSUM):
  - Startup: x rides the sync+scalar DMA queues in 3 grouped transfers
    (single-chunk strided transfers measured only ~30GB/s; grouped 5-12KB
    lines are fast); weights/biases ride gpsimd.  K/V/Q consume x chunks in
    arrival order with chunk-paired PSUM accumulation, so the PE starts
    ~8us in and is DMA-paced only briefly.
  - Attention: 3-stage pipeline over 32 (window, kv-group, head-pair)
    units, window-major.  S^T on PE, one [128,1024] exp on ACT, colsum via
    ones-matmul, broadcast via K=1 matmul, reciprocal+scale on DVE.
  - As soon as a window's 8 pairs retire, the out-projection rows of that
    window (16 matmuls each) are interleaved into the pipeline, so the PE
    has dense filler while ACT/DVE round-trip, and the out-projection
    effectively starts ~40us early.
All weights are host-pretiled so every DMA is a large contiguous transfer.
"""

import numpy as np
import ml_dtypes

import concourse.bacc as bacc
import concourse.tile as tile
from concourse import mybir
from concourse.bass_utils import run_bass_kernel_spmd

F32 = mybir.dt.float32
BF16 = mybir.dt.bfloat16

B = 2
L = 4096
C = 2048          # embed dim
H = 16            # heads
G = 2             # kv groups
D = 128           # head dim
KV = G * D        # 256
NWL = 4           # windows per core
S = NWL * 128 + 128   # 640 input positions per core
OUT_ROWS = NWL * 256  # 1024 output rows per core
KC = C // 128     # 16 contraction chunks
NT = 4            # out-proj 512-col tiles
SC_OUT = OUT_ROWS // 128
SCALE = 1.0 / float(np.sqrt(D))
N_CORES = 8

# Q-projection head order: pair (g, j) covers heads (g+4j, g+4j+2)
HEAD_ORDER = [h for j in range(4) for g in range(G)
              for h in (g + 4 * j, g + 4 * j + 2)]

_CACHE = {}


def _build():
    nc = bacc.Bacc(None, target_bir_lowering=False)

    # host-pretiled layouts (see kernel() for the numpy side)
    xT_d = nc.dram_tensor("xT", [128, KC, S], BF16, kind="ExternalInput")
    wq_d = nc.dram_tensor("Wq", [H, 128, KC, 128], BF16, kind="ExternalInput")
    wk_d = nc.dram_tensor("Wk", [G, 128, KC, 128], BF16, kind="ExternalInput")
    wv_d = nc.dram_tensor("Wv", [128, KC, KV], BF16, kind="ExternalInput")
    wo_d = nc.dram_tensor("Wo", [NT, 128, KC, 512], BF16, kind="ExternalInput")
    bq_d = nc.dram_tensor("bq", [128, H], F32, kind="ExternalInput")
    bk_d = nc.dram_tensor("bk", [128, G], F32, kind="ExternalInput")
    bv_d = nc.dram_tensor("bv", [128, KV], F32, kind="ExternalInput")
    bo_d = nc.dram_tensor("bo", [128, C], F32, kind="ExternalInput")
    out_d = nc.dram_tensor("out", [NT, SC_OUT, 128, 512], F32,
                           kind="ExternalOutput")

    NA = 320  # free split of S=640 (psum bank = 512 f32)

    with tile.TileContext(nc) as tc:
        with (
            tc.tile_pool(name="res", bufs=1) as res,
            tc.tile_pool(name="wqs", bufs=3) as wqs,
            tc.tile_pool(name="wos", bufs=2) as wos,
            tc.tile_pool(name="work", bufs=4) as work,
            tc.tile_pool(name="norm", bufs=3) as norm,
        ):
            # ---------- resident tiles + input DMAs ----------
            # x in 3 grouped DMAs: two on sync, one on the scalar queue
            # (parallel rings), consumed in arrival order
            XG = [(0, 1, "sync"), (1, 6, "sync"), (6, 11, "sync"), (11, 16, "sync")]
            xg = [res.tile([128, hi - lo, S], BF16, tag=f"xg{i}",
                           name=f"xg{i}")
                  for i, (lo, hi, _) in enumerate(XG)]

            def xts(kc):
                for i, (lo, hi, _) in enumerate(XG):
                    if lo <= kc < hi:
                        return xg[i][:, kc - lo, :]
                raise AssertionError(kc)

            kw = [res.tile([128, KC, 128], BF16, tag=f"kw{g}", name=f"kw{g}")
                  for g in range(G)]
            wv_t = res.tile([128, KC, KV], BF16, tag="wv", name="wv")
            bq_sb = res.tile([128, H], F32, tag="bq", name="bq")
            bk_sb = res.tile([128, G], F32, tag="bk", name="bk")
            bv_bc = res.tile([128, KV], F32, tag="bvbc", name="bvbc")
            bo_bc = res.tile([128, C], F32, tag="bobc", name="bobc")

            # tiny bias DMAs FIRST on gpsimd (behind big transfers they
            # arrive 25us late and stall the first ACT drains)
            nc.gpsimd.dma_start(out=bq_sb, in_=bq_d[:, :])
            nc.gpsimd.dma_start(out=bk_sb, in_=bk_d[:, :])
            nc.gpsimd.dma_start(out=bv_bc, in_=bv_d[:, :])
            for i, (lo, hi, q) in enumerate(XG):
                eng = nc.sync if q == "sync" else nc.scalar
                eng.dma_start(out=xg[i], in_=xT_d[:, lo:hi, :])
            for g in range(G):
                nc.scalar.dma_start(out=kw[g], in_=wk_d[g])
            nc.scalar.dma_start(out=wv_t, in_=wv_d[:, :, :])
            # bo broadcast (1MB SBUF write) deferred off the startup path

            qp = [res.tile([128, 2, S], BF16, tag=f"qp{i}", name=f"qp{i}")
                  for i in range(8)]

            def q_slot(h):
                g, k = h % G, h // G
                return qp[g * 4 + k // 2][:, k % 2, :]

            kt = [res.tile([128, S], BF16, tag=f"kt{g}", name=f"kt{g}")
                  for g in range(G)]
            vt = [res.tile([128, KV], BF16, tag=f"vt{sc}", name=f"vt{sc}")
                  for sc in range(S // 128)]
            ot = [res.tile([128, OUT_ROWS], BF16, tag=f"ot{h}", name=f"ot{h}")
                  for h in range(H)]

            ones = res.tile([128, 1], BF16, tag="ones", name="ones")
            nc.vector.memset(ones, 1.0)
            ones_r = res.tile([1, 128], BF16, tag="ones_r", name="ones_r")
            nc.vector.memset(ones_r, 1.0)

            # wq prefetch, depth 3 (first two heads ride gpsimd so they do
            # not serialize behind the x stream on sync)
            wq_pending = {}

            def prefetch_wq(h, queue=None):
                t = wqs.tile([128, KC, 128], BF16, tag="wq", name="wq")
                (queue or nc.sync).dma_start(out=t, in_=wq_d[h])
                wq_pending[h] = t

            prefetch_wq(HEAD_ORDER[0], queue=nc.gpsimd)
            prefetch_wq(HEAD_ORDER[1], queue=nc.gpsimd)
            WQ_QUEUES = {}
            for _i in range(2, H):
                WQ_QUEUES[HEAD_ORDER[_i]] = nc.sync if _i % 2 == 0 else nc.scalar

            # ---------- projections (chunk-paired, DMA-paced) ----------
            with tc.tile_pool(name="psQ", bufs=2, space="PSUM") as psQ:
                KORD = list(range(KC))
                for g in range(G):
                    pa = psQ.tile([128, NA], F32, tag="qa", name="ka")
                    pb = psQ.tile([128, NA], F32, tag="qb", name="kb")
                    for ki, kc in enumerate(KORD):
                        nc.tensor.matmul(pa, lhsT=kw[g][:, kc, :],
                                         rhs=xts(kc)[:, 0:NA],
                                         start=(ki == 0), stop=(ki == KC - 1))
                        nc.tensor.matmul(pb, lhsT=kw[g][:, kc, :],
                                         rhs=xts(kc)[:, NA:S],
                                         start=(ki == 0), stop=(ki == KC - 1))
                    nc.scalar.activation(kt[g][:, 0:NA], pa,
                                         mybir.ActivationFunctionType.Identity,
                                         bias=bk_sb[:, g:g + 1])
                    nc.scalar.activation(kt[g][:, NA:S], pb,
                                         mybir.ActivationFunctionType.Identity,
                                         bias=bk_sb[:, g:g + 1])

                for sc in range(S // 128):
                    pv = psQ.tile([128, KV], F32,
                                  tag=("qa" if sc % 2 == 0 else "qb"),
                                  name="pv")
                    for kc in range(KC):
                        nc.tensor.matmul(
                            pv, lhsT=xts(kc)[:, sc * 128:(sc + 1) * 128],
                            rhs=wv_t[:, kc, :],
                            start=(kc == 0), stop=(kc == KC - 1))
                    nc.vector.tensor_add(vt[sc], pv, bv_bc)

                for idx, h in enumerate(HEAD_ORDER):
                    if idx + 2 < H:
                        hh = HEAD_ORDER[idx + 2]
                        prefetch_wq(hh, queue=WQ_QUEUES[hh])
                    wq_t = wq_pending.pop(h)
                    pa = psQ.tile([128, NA], F32, tag="qa", name="qa")
                    pb = psQ.tile([128, NA], F32, tag="qb", name="qb")
                    for kc in range(KC):
                        nc.tensor.matmul(pa, lhsT=wq_t[:, kc, :],
                                         rhs=xts(kc)[:, 0:NA],
                                         start=(kc == 0), stop=(kc == KC - 1))
                        nc.tensor.matmul(pb, lhsT=wq_t[:, kc, :],
                                         rhs=xts(kc)[:, NA:S],
                                         start=(kc == 0), stop=(kc == KC - 1))
                    nc.scalar.activation(q_slot(h)[:, 0:NA], pa,
                                         mybir.ActivationFunctionType.Identity,
                                         bias=bq_sb[:, h:h + 1])
                    nc.scalar.activation(q_slot(h)[:, NA:S], pb,
                                         mybir.ActivationFunctionType.Identity,
                                         bias=bq_sb[:, h:h + 1])

            # ---------- attention + early out-projection ----------
            # pairs window-major: window w's ot columns complete after its 8
            # pairs, unlocking out-proj rows 2w, 2w+1 of block nt=0.
            pairs = [(w, g, j) for w in range(NWL)
                     for g in range(G) for j in range(4)]
            NP = len(pairs)
            state = {}
            wo_tiles = {}

            def prefetch_wo(nt):
                if nt == 0:
                    nc.gpsimd.dma_start(out=bo_bc, in_=bo_d[:, :])
                t = wos.tile([128, KC, 512], BF16, tag="wo", name="wo")
                nc.sync.dma_start(out=t, in_=wo_d[nt])
                wo_tiles[nt] = t

            def po_unit(nt, sc):
                po = psB2.tile([128, 512], F32, tag="ob", name="po")
                for fc in range(KC):
                    nc.tensor.matmul(
                        po, lhsT=ot[fc][:, sc * 128:(sc + 1) * 128],
                        rhs=wo_tiles[nt][:, fc, :],
                        start=(fc == 0), stop=(fc == KC - 1))
                osb = work.tile([128, 512], F32, tag="osb", name="osb")
                nc.vector.tensor_add(osb, po,
                                     bo_bc[:, nt * 512:(nt + 1) * 512])
                nc.sync.dma_start(out=out_d[nt, sc], in_=osb)

            def stage_a(i):
                w, g, j = pairs[i]
                qpt = qp[g * 4 + j]
                stt = psB.tile([128, 1024], F32, tag="st", name="stt")
                for kc in range(2):
                    nc.tensor.matmul(
                        stt[:, kc * 512:(kc + 1) * 512],
                        lhsT=kt[g][:, (w + kc) * 128:(w + kc + 1) * 128],
                        rhs=qpt[:, :, w * 128:w * 128 + 256],
                        start=True, stop=True)
                pt = work.tile([128, 1024], BF16, tag="pt", name="pt")
                nc.scalar.activation(pt, stt,
                                     mybir.ActivationFunctionType.Exp,
                                     scale=SCALE)
                state[i] = [pt]

            def stage_b(i):
                w, g, j = pairs[i]
                (pt,) = state[i]
                obt = psB2.tile([128, 512], F32, tag="ob", name="obt")
                for kc in range(2):
                    nc.tensor.matmul(
                        obt, lhsT=vt[w + kc][:, g * 128:(g + 1) * 128],
                        rhs=pt[:, kc * 512:(kc + 1) * 512],
                        start=(kc == 0), stop=(kc == 1))
                cs = psB3.tile([1, 512], F32, tag="cb", name="cs")
                for kc in range(2):
                    nc.tensor.matmul(cs, lhsT=ones,
                                     rhs=pt[:, kc * 512:(kc + 1) * 512],
                                     start=(kc == 0), stop=(kc == 1))
                csb = norm.tile([1, 512], BF16, tag="csb", name="csb")
                nc.scalar.copy(csb, cs)
                state[i] = [obt, csb]

            def stage_c(i):
                w, g, j = pairs[i]
                h0, h1 = g + 4 * j, g + 4 * j + 2
                obt, csb = state.pop(i)
                bc = psB3.tile([128, 512], F32, tag="cb", name="bc")
                nc.tensor.matmul(bc, lhsT=ones_r, rhs=csb,
                                 start=True, stop=True)
                bcr = norm.tile([128, 512], F32, tag="bcr", name="bcr")
                nc.vector.reciprocal_approx_fast(out=bcr, in_=bc)
                ws = slice(w * 256, (w + 1) * 256)
                nc.vector.tensor_mul(ot[h0][:, ws], obt[:, 0:256],
                                     bcr[:, 0:256])
                nc.vector.tensor_mul(ot[h1][:, ws], obt[:, 256:512],
                                     bcr[:, 256:512])

            with (
                tc.tile_pool(name="psB", bufs=2, space="PSUM") as psB,
                tc.tile_pool(name="psB2", bufs=2, space="PSUM") as psB2,
                tc.tile_pool(name="psB3", bufs=2, space="PSUM") as psB3,
            ):
                prefetch_wo(0)
                for i in range(NP + 4):
                    if 2 <= i < NP + 2:
                        stage_b(i - 2)
                    if i < NP:
                        stage_a(i)
                    if 4 <= i:
                        stage_c(i - 4)
                        if (i - 4) % 8 == 7:       # window w fully retired
                            w = (i - 4) // 8
                            po_unit(0, 2 * w)
                            if w == 0:
                                prefetch_wo(1)
                            po_unit(0, 2 * w + 1)

                # remaining out-proj blocks
                for nt in range(1, NT):
                    if nt + 1 < NT:
                        prefetch_wo(nt + 1)
                    for sc in range(SC_OUT):
                        po_unit(nt, sc)

    nc.compile()
    return nc


def _get_nc():
    if "nc" not in _CACHE:
        _CACHE["nc"] = _build()
    return _CACHE["nc"]


def _prep_weights(Wq, bq, Wk, bk, Wv, bv, Wo, bo):
    bf16 = ml_dtypes.bfloat16
    f32 = lambda a: np.ascontiguousarray(np.asarray(a, dtype=np.float32))
    wq = np.asarray(Wq, np.float32).reshape(KC, 128, H, 128)
    wq = np.ascontiguousarray(wq.transpose(2, 1, 0, 3)).astype(bf16)  # (H,p,kc,f)
    wk = np.asarray(Wk, np.float32).reshape(KC, 128, G, 128)
    wk = np.ascontiguousarray(wk.transpose(2, 1, 0, 3)).astype(bf16)  # (G,p,kc,f)
    wv = np.asarray(Wv, np.float32).reshape(KC, 128, KV)
    wv = np.ascontiguousarray(wv.transpose(1, 0, 2)).astype(bf16)     # (p,kc,f)
    wo = np.asarray(Wo, np.float32).reshape(KC, 128, NT, 512)
    wo = np.ascontiguousarray(wo.transpose(2, 1, 0, 3)).astype(bf16)  # (NT,p,kc,f)
    bq2 = np.ascontiguousarray(f32(bq).reshape(H, 128).T)   # (p, h)
    bk2 = np.ascontiguousarray(f32(bk).reshape(G, 128).T)    # (p, g)
    # bv/bo pre-broadcast on host: a [1,N]->[128,N] broadcast DMA lowers to
    # per-element packets (~20us); a contiguous 2D copy is ~2us
    bv2 = np.ascontiguousarray(np.broadcast_to(f32(bv), (128, KV)))
    bo2 = np.ascontiguousarray(np.broadcast_to(f32(bo), (128, C)))
    return {
        "Wq": wq, "Wk": wk, "Wv": wv, "Wo": wo,
        "bq": bq2, "bk": bk2, "bv": bv2, "bo": bo2,
    }


def _prep_in_maps(x, weights):
    bf16 = ml_dtypes.bfloat16
    in_maps = []
    for c in range(N_CORES):
        b, p = divmod(c, 4)
        xs = np.asarray(x[b, 512 * p:512 * p + S, :], np.float32)
        xT = np.ascontiguousarray(xs.T.reshape(KC, 128, S).transpose(1, 0, 2))
        in_maps.append(dict(weights, xT=xT.astype(bf16)))
    return in_maps


def kernel(x, Wq, bq, Wk, bk, Wv, bv, Wo, bo, **_):
    x = np.asarray(x, dtype=np.float32)
    weights = _prep_weights(Wq, bq, Wk, bk, Wv, bv, Wo, bo)
    in_maps = _prep_in_maps(x, weights)

    nc = _get_nc()
    res = run_bass_kernel_spmd(nc, in_maps, core_ids=list(range(N_CORES)))

    out = np.empty((B, L, C), dtype=np.float32)
    for c in range(N_CORES):
        b, p = divmod(c, 4)
        blk = res.results[c]["out"]  # (NT, SC_OUT, 128, 512)
        rows = blk.transpose(1, 2, 0, 3).reshape(OUT_ROWS, C)
        out[b, 1024 * p:1024 * p + OUT_ROWS, :] = rows
    return out


# revision 26
# speedup vs baseline: 1.0597x; 1.0028x over previous
"""Trainium2 Bass kernel for EnhancedLocalAttentionWithGQA.

Problem (hardcoded): B=2, L=4096, C=2048, H=16 heads, D=128, G=2 kv groups,
window W=256 with stride 128 (50% overlap).

Key observation: the reference computes NW=31 overlapping windows but the
final output slice [:, :L] keeps only windows 0..15 (16 windows x 256 rows
= 4096 rows).  Window n's output rows [n*256,(n+1)*256) come from queries /
keys / values at input positions [n*128, n*128+256).  So only x positions
0..2175 feed QKV, and each window is an independent 256x256 attention.

Sharding (8 cores): core c -> batch b=c//4, quarter p=c%4, i.e. 4 windows
(global windows 4p..4p+3), input positions [512p, 512p+640), output rows
[1024p, 1024p+1024) of batch b.  No collectives; host concatenates rows.

Per-core pipeline (bf16 matmuls, fp32 PSUM):
  - Startup: x rides the sync+scalar DMA queues in 3 grouped transfers
    (single-chunk strided transfers measured only ~30GB/s; grouped 5-12KB
    lines are fast); weights/biases ride gpsimd.  K/V/Q consume x chunks in
    arrival order with chunk-paired PSUM accumulation, so the PE starts
    ~8us in and is DMA-paced only briefly.
  - Attention: 3-stage pipeline over 32 (window, kv-group, head-pair)
    units, window-major.  S^T on PE, one [128,1024] exp on ACT, colsum via
    ones-matmul, broadcast via K=1 matmul, reciprocal+scale on DVE.
  - As soon as a window's 8 pairs retire, the out-projection rows of that
    window (16 matmuls each) are interleaved into the pipeline, so the PE
    has dense filler while ACT/DVE round-trip, and the out-projection
    effectively starts ~40us early.
All weights are host-pretiled so every DMA is a large contiguous transfer.
"""

import numpy as np
import ml_dtypes

import concourse.bacc as bacc
import concourse.tile as tile
from concourse import mybir
from concourse.bass_utils import run_bass_kernel_spmd

F32 = mybir.dt.float32
BF16 = mybir.dt.bfloat16

B = 2
L = 4096
C = 2048          # embed dim
H = 16            # heads
G = 2             # kv groups
D = 128           # head dim
KV = G * D        # 256
NWL = 4           # windows per core
S = NWL * 128 + 128   # 640 input positions per core
OUT_ROWS = NWL * 256  # 1024 output rows per core
KC = C // 128     # 16 contraction chunks
NT = 4            # out-proj 512-col tiles
SC_OUT = OUT_ROWS // 128
SCALE = 1.0 / float(np.sqrt(D))
N_CORES = 8

# Q-projection head order: pair (g, j) covers heads (g+4j, g+4j+2)
HEAD_ORDER = [h for j in range(4) for g in range(G)
              for h in (g + 4 * j, g + 4 * j + 2)]

_CACHE = {}


def _build():
    nc = bacc.Bacc(None, target_bir_lowering=False)

    # host-pretiled layouts (see kernel() for the numpy side)
    xT_d = nc.dram_tensor("xT", [128, KC, S], BF16, kind="ExternalInput")
    wq_d = nc.dram_tensor("Wq", [H, 128, KC, 128], BF16, kind="ExternalInput")
    wk_d = nc.dram_tensor("Wk", [G, 128, KC, 128], BF16, kind="ExternalInput")
    wv_d = nc.dram_tensor("Wv", [128, KC, KV], BF16, kind="ExternalInput")
    wo_d = nc.dram_tensor("Wo", [NT, 128, KC, 512], BF16, kind="ExternalInput")
    bq_d = nc.dram_tensor("bq", [128, H], F32, kind="ExternalInput")
    bk_d = nc.dram_tensor("bk", [128, G], F32, kind="ExternalInput")
    bv_d = nc.dram_tensor("bv", [128, KV], F32, kind="ExternalInput")
    bo_d = nc.dram_tensor("bo", [128, C], F32, kind="ExternalInput")
    out_d = nc.dram_tensor("out", [NT, SC_OUT, 128, 512], F32,
                           kind="ExternalOutput")

    NA = 320  # free split of S=640 (psum bank = 512 f32)

    with tile.TileContext(nc) as tc:
        with (
            tc.tile_pool(name="res", bufs=1) as res,
            tc.tile_pool(name="wqs", bufs=3) as wqs,
            tc.tile_pool(name="wos", bufs=2) as wos,
            tc.tile_pool(name="work", bufs=4) as work,
            tc.tile_pool(name="norm", bufs=3) as norm,
        ):
            # ---------- resident tiles + input DMAs ----------
            # x in 3 grouped DMAs: two on sync, one on the scalar queue
            # (parallel rings), consumed in arrival order
            XG = [(0, 1, "sync"), (1, 4, "sync"), (4, 8, "sync"), (8, 12, "sync"), (12, 16, "sync")]
            xg = [res.tile([128, hi - lo, S], BF16, tag=f"xg{i}",
                           name=f"xg{i}")
                  for i, (lo, hi, _) in enumerate(XG)]

            def xts(kc):
                for i, (lo, hi, _) in enumerate(XG):
                    if lo <= kc < hi:
                        return xg[i][:, kc - lo, :]
                raise AssertionError(kc)

            kw = [res.tile([128, KC, 128], BF16, tag=f"kw{g}", name=f"kw{g}")
                  for g in range(G)]
            wv_t = res.tile([128, KC, KV], BF16, tag="wv", name="wv")
            bq_sb = res.tile([128, H], F32, tag="bq", name="bq")
            bk_sb = res.tile([128, G], F32, tag="bk", name="bk")
            bv_bc = res.tile([128, KV], F32, tag="bvbc", name="bvbc")
            bo_bc = res.tile([128, C], F32, tag="bobc", name="bobc")

            # tiny bias DMAs FIRST on gpsimd (behind big transfers they
            # arrive 25us late and stall the first ACT drains)
            nc.gpsimd.dma_start(out=bq_sb, in_=bq_d[:, :])
            nc.gpsimd.dma_start(out=bk_sb, in_=bk_d[:, :])
            nc.gpsimd.dma_start(out=bv_bc, in_=bv_d[:, :])
            for i, (lo, hi, q) in enumerate(XG):
                eng = nc.sync if q == "sync" else nc.scalar
                eng.dma_start(out=xg[i], in_=xT_d[:, lo:hi, :])
            # kw0 split so chunk 0 lands fast (subtile deps let K start)
            nc.scalar.dma_start(out=kw[0][:, 0:4, :], in_=wk_d[0][:, 0:4, :])
            nc.scalar.dma_start(out=kw[0][:, 4:KC, :], in_=wk_d[0][:, 4:KC, :])
            nc.scalar.dma_start(out=kw[1], in_=wk_d[1])
            nc.scalar.dma_start(out=wv_t, in_=wv_d[:, :, :])
            # bo broadcast (1MB SBUF write) deferred off the startup path

            qp = [res.tile([128, 2, S], BF16, tag=f"qp{i}", name=f"qp{i}")
                  for i in range(8)]

            def q_slot(h):
                g, k = h % G, h // G
                return qp[g * 4 + k // 2][:, k % 2, :]

            kt = [res.tile([128, S], BF16, tag=f"kt{g}", name=f"kt{g}")
                  for g in range(G)]
            vt = [res.tile([128, KV], BF16, tag=f"vt{sc}", name=f"vt{sc}")
                  for sc in range(S // 128)]
            ot = [res.tile([128, OUT_ROWS], BF16, tag=f"ot{h}", name=f"ot{h}")
                  for h in range(H)]

            ones = res.tile([128, 1], BF16, tag="ones", name="ones")
            nc.vector.memset(ones, 1.0)
            ones_r = res.tile([1, 128], BF16, tag="ones_r", name="ones_r")
            nc.vector.memset(ones_r, 1.0)

            # wq prefetch, depth 3 (first two heads ride gpsimd so they do
            # not serialize behind the x stream on sync)
            wq_pending = {}

            def prefetch_wq(h, queue=None):
                t = wqs.tile([128, KC, 128], BF16, tag="wq", name="wq")
                (queue or nc.sync).dma_start(out=t, in_=wq_d[h])
                wq_pending[h] = t

            prefetch_wq(HEAD_ORDER[0], queue=nc.gpsimd)
            prefetch_wq(HEAD_ORDER[1], queue=nc.gpsimd)
            WQ_QUEUES = {}
            for _i in range(2, H):
                WQ_QUEUES[HEAD_ORDER[_i]] = nc.sync if _i % 2 == 0 else nc.scalar

            # ---------- projections (chunk-paired, DMA-paced) ----------
            with tc.tile_pool(name="psQ", bufs=2, space="PSUM") as psQ:
                KORD = list(range(KC))
                for g in range(G):
                    pa = psQ.tile([128, NA], F32, tag="qa", name="ka")
                    pb = psQ.tile([128, NA], F32, tag="qb", name="kb")
                    for ki, kc in enumerate(KORD):
                        nc.tensor.matmul(pa, lhsT=kw[g][:, kc, :],
                                         rhs=xts(kc)[:, 0:NA],
                                         start=(ki == 0), stop=(ki == KC - 1))
                        nc.tensor.matmul(pb, lhsT=kw[g][:, kc, :],
                                         rhs=xts(kc)[:, NA:S],
                                         start=(ki == 0), stop=(ki == KC - 1))
                    nc.scalar.activation(kt[g][:, 0:NA], pa,
                                         mybir.ActivationFunctionType.Identity,
                                         bias=bk_sb[:, g:g + 1])
                    nc.scalar.activation(kt[g][:, NA:S], pb,
                                         mybir.ActivationFunctionType.Identity,
                                         bias=bk_sb[:, g:g + 1])

                for sc in range(S // 128):
                    pv = psQ.tile([128, KV], F32,
                                  tag=("qa" if sc % 2 == 0 else "qb"),
                                  name="pv")
                    for kc in range(KC):
                        nc.tensor.matmul(
                            pv, lhsT=xts(kc)[:, sc * 128:(sc + 1) * 128],
                            rhs=wv_t[:, kc, :],
                            start=(kc == 0), stop=(kc == KC - 1))
                    nc.vector.tensor_add(vt[sc], pv, bv_bc)

                for idx, h in enumerate(HEAD_ORDER):
                    if idx + 2 < H:
                        hh = HEAD_ORDER[idx + 2]
                        prefetch_wq(hh, queue=WQ_QUEUES[hh])
                    wq_t = wq_pending.pop(h)
                    pa = psQ.tile([128, NA], F32, tag="qa", name="qa")
                    pb = psQ.tile([128, NA], F32, tag="qb", name="qb")
                    for kc in range(KC):
                        nc.tensor.matmul(pa, lhsT=wq_t[:, kc, :],
                                         rhs=xts(kc)[:, 0:NA],
                                         start=(kc == 0), stop=(kc == KC - 1))
                        nc.tensor.matmul(pb, lhsT=wq_t[:, kc, :],
                                         rhs=xts(kc)[:, NA:S],
                                         start=(kc == 0), stop=(kc == KC - 1))
                    nc.scalar.activation(q_slot(h)[:, 0:NA], pa,
                                         mybir.ActivationFunctionType.Identity,
                                         bias=bq_sb[:, h:h + 1])
                    nc.scalar.activation(q_slot(h)[:, NA:S], pb,
                                         mybir.ActivationFunctionType.Identity,
                                         bias=bq_sb[:, h:h + 1])

            # ---------- attention + early out-projection ----------
            # pairs window-major: window w's ot columns complete after its 8
            # pairs, unlocking out-proj rows 2w, 2w+1 of block nt=0.
            pairs = [(w, g, j) for w in range(NWL)
                     for g in range(G) for j in range(4)]
            NP = len(pairs)
            state = {}
            wo_tiles = {}

            def prefetch_wo(nt):
                if nt == 0:
                    nc.gpsimd.dma_start(out=bo_bc, in_=bo_d[:, :])
                t = wos.tile([128, KC, 512], BF16, tag="wo", name="wo")
                nc.sync.dma_start(out=t, in_=wo_d[nt])
                wo_tiles[nt] = t

            def po_unit(nt, sc):
                po = psB2.tile([128, 512], F32, tag="ob", name="po")
                for fc in range(KC):
                    nc.tensor.matmul(
                        po, lhsT=ot[fc][:, sc * 128:(sc + 1) * 128],
                        rhs=wo_tiles[nt][:, fc, :],
                        start=(fc == 0), stop=(fc == KC - 1))
                osb = work.tile([128, 512], F32, tag="osb", name="osb")
                nc.vector.tensor_add(osb, po,
                                     bo_bc[:, nt * 512:(nt + 1) * 512])
                nc.sync.dma_start(out=out_d[nt, sc], in_=osb)

            def stage_a(i):
                w, g, j = pairs[i]
                qpt = qp[g * 4 + j]
                stt = psB.tile([128, 1024], F32, tag="st", name="stt")
                for kc in range(2):
                    nc.tensor.matmul(
                        stt[:, kc * 512:(kc + 1) * 512],
                        lhsT=kt[g][:, (w + kc) * 128:(w + kc + 1) * 128],
                        rhs=qpt[:, :, w * 128:w * 128 + 256],
                        start=True, stop=True)
                pt = work.tile([128, 1024], BF16, tag="pt", name="pt")
                nc.scalar.activation(pt, stt,
                                     mybir.ActivationFunctionType.Exp,
                                     scale=SCALE)
                state[i] = [pt]

            def stage_b(i):
                w, g, j = pairs[i]
                (pt,) = state[i]
                obt = psB2.tile([128, 512], F32, tag="ob", name="obt")
                for kc in range(2):
                    nc.tensor.matmul(
                        obt, lhsT=vt[w + kc][:, g * 128:(g + 1) * 128],
                        rhs=pt[:, kc * 512:(kc + 1) * 512],
                        start=(kc == 0), stop=(kc == 1))
                cs = psB3.tile([1, 512], F32, tag="cb", name="cs")
                for kc in range(2):
                    nc.tensor.matmul(cs, lhsT=ones,
                                     rhs=pt[:, kc * 512:(kc + 1) * 512],
                                     start=(kc == 0), stop=(kc == 1))
                csb = norm.tile([1, 512], BF16, tag="csb", name="csb")
                nc.scalar.copy(csb, cs)
                state[i] = [obt, csb]

            def stage_c(i):
                w, g, j = pairs[i]
                h0, h1 = g + 4 * j, g + 4 * j + 2
                obt, csb = state.pop(i)
                bc = psB3.tile([128, 512], F32, tag="cb", name="bc")
                nc.tensor.matmul(bc, lhsT=ones_r, rhs=csb,
                                 start=True, stop=True)
                bcr = norm.tile([128, 512], F32, tag="bcr", name="bcr")
                nc.vector.reciprocal_approx_fast(out=bcr, in_=bc)
                ws = slice(w * 256, (w + 1) * 256)
                nc.vector.tensor_mul(ot[h0][:, ws], obt[:, 0:256],
                                     bcr[:, 0:256])
                nc.vector.tensor_mul(ot[h1][:, ws], obt[:, 256:512],
                                     bcr[:, 256:512])

            with (
                tc.tile_pool(name="psB", bufs=2, space="PSUM") as psB,
                tc.tile_pool(name="psB2", bufs=2, space="PSUM") as psB2,
                tc.tile_pool(name="psB3", bufs=2, space="PSUM") as psB3,
            ):
                prefetch_wo(0)
                for i in range(NP + 4):
                    if 2 <= i < NP + 2:
                        stage_b(i - 2)
                    if i < NP:
                        stage_a(i)
                    if 4 <= i:
                        stage_c(i - 4)
                        if (i - 4) % 8 == 7:       # window w fully retired
                            w = (i - 4) // 8
                            po_unit(0, 2 * w)
                            if w == 0:
                                prefetch_wo(1)
                            po_unit(0, 2 * w + 1)

                # remaining out-proj blocks
                for nt in range(1, NT):
                    if nt + 1 < NT:
                        prefetch_wo(nt + 1)
                    for sc in range(SC_OUT):
                        po_unit(nt, sc)

    nc.compile()
    return nc


def _get_nc():
    if "nc" not in _CACHE:
        _CACHE["nc"] = _build()
    return _CACHE["nc"]


def _prep_weights(Wq, bq, Wk, bk, Wv, bv, Wo, bo):
    bf16 = ml_dtypes.bfloat16
    f32 = lambda a: np.ascontiguousarray(np.asarray(a, dtype=np.float32))
    wq = np.asarray(Wq, np.float32).reshape(KC, 128, H, 128)
    wq = np.ascontiguousarray(wq.transpose(2, 1, 0, 3)).astype(bf16)  # (H,p,kc,f)
    wk = np.asarray(Wk, np.float32).reshape(KC, 128, G, 128)
    wk = np.ascontiguousarray(wk.transpose(2, 1, 0, 3)).astype(bf16)  # (G,p,kc,f)
    wv = np.asarray(Wv, np.float32).reshape(KC, 128, KV)
    wv = np.ascontiguousarray(wv.transpose(1, 0, 2)).astype(bf16)     # (p,kc,f)
    wo = np.asarray(Wo, np.float32).reshape(KC, 128, NT, 512)
    wo = np.ascontiguousarray(wo.transpose(2, 1, 0, 3)).astype(bf16)  # (NT,p,kc,f)
    bq2 = np.ascontiguousarray(f32(bq).reshape(H, 128).T)   # (p, h)
    bk2 = np.ascontiguousarray(f32(bk).reshape(G, 128).T)    # (p, g)
    # bv/bo pre-broadcast on host: a [1,N]->[128,N] broadcast DMA lowers to
    # per-element packets (~20us); a contiguous 2D copy is ~2us
    bv2 = np.ascontiguousarray(np.broadcast_to(f32(bv), (128, KV)))
    bo2 = np.ascontiguousarray(np.broadcast_to(f32(bo), (128, C)))
    return {
        "Wq": wq, "Wk": wk, "Wv": wv, "Wo": wo,
        "bq": bq2, "bk": bk2, "bv": bv2, "bo": bo2,
    }


def _prep_in_maps(x, weights):
    bf16 = ml_dtypes.bfloat16
    in_maps = []
    for c in range(N_CORES):
        b, p = divmod(c, 4)
        xs = np.asarray(x[b, 512 * p:512 * p + S, :], np.float32)
        xT = np.ascontiguousarray(xs.T.reshape(KC, 128, S).transpose(1, 0, 2))
        in_maps.append(dict(weights, xT=xT.astype(bf16)))
    return in_maps


def kernel(x, Wq, bq, Wk, bk, Wv, bv, Wo, bo, **_):
    x = np.asarray(x, dtype=np.float32)
    weights = _prep_weights(Wq, bq, Wk, bk, Wv, bv, Wo, bo)
    in_maps = _prep_in_maps(x, weights)

    nc = _get_nc()
    res = run_bass_kernel_spmd(nc, in_maps, core_ids=list(range(N_CORES)))

    out = np.empty((B, L, C), dtype=np.float32)
    for c in range(N_CORES):
        b, p = divmod(c, 4)
        blk = res.results[c]["out"]  # (NT, SC_OUT, 128, 512)
        rows = blk.transpose(1, 2, 0, 3).reshape(OUT_ROWS, C)
        out[b, 1024 * p:1024 * p + OUT_ROWS, :] = rows
    return out
